# revision 1
# baseline (speedup 1.0000x reference)
"""GCN (4-layer, message passing) on 8 Trainium2 NeuronCores via Bass/Tile.

Sharding: pure data parallelism over graphs (32 graphs / core via the sorted
`batch` vector). Each core owns its graphs' nodes (re-permuted into
degree-balanced 128-node tiles) and all edges whose *destination* lands on it.

Per layer:
  AllGather(h)  ->  indirect-DMA gather of edge-source rows (bf16)
  -> segment-sum via one-hot matmuls on the TensorEngine (S precomputed host-side)
  -> transform agg @ W_l + b_l (bias via ones-row matmul), ReLU on ScalarE.
Uses the GCN linearity segsum(h@W) == segsum(h)@W to aggregate raw h.

Mean-pool = matmul with 0/1 pool matrix + fp32 inv-count scale; 3-layer MLP
on device; per-core [32, 10] outputs concatenated on the host.
"""
import numpy as np
import ml_dtypes

import concourse.bass as bass
import concourse.tile as tile
from concourse import bacc, mybir
from concourse.bass import IndirectOffsetOnAxis
from concourse.bass_utils import run_bass_kernel_spmd
from concourse.masks import make_identity

P = 128
D = 146
DH = 73  # D // 2
N_LAYERS = 4
N_GRAPHS = 256
F32 = mybir.dt.float32
BF16 = mybir.dt.bfloat16
I32 = mybir.dt.int32
BF = ml_dtypes.bfloat16
FP8 = ml_dtypes.float8_e4m3


# ----------------------------------------------------------------- host prep
def _prep(x, edge_index, batch, ncores):
    """Shard nodes by graph block, re-permute into degree-balanced tiles,
    build per-core gather offsets + one-hot S chunks + pool matrices."""
    n_nodes = x.shape[0]
    gp = N_GRAPHS // ncores  # graphs per core
    batch = np.asarray(batch, np.int64)
    core_of_node = (batch // gp).astype(np.int64)
    # contiguous node range per core (batch is sorted)
    n0 = np.searchsorted(core_of_node, np.arange(ncores), side="left")
    n1 = np.searchsorted(core_of_node, np.arange(ncores), side="right")
    cnt = n1 - n0
    nshard = int(np.ceil(cnt.max() / P) * P)
    nt = nshard // P

    src_g = np.asarray(edge_index[0], np.int64)
    dst_g = np.asarray(edge_index[1], np.int64)

    # in-degree per node (global)
    deg = np.bincount(dst_g, minlength=n_nodes)

    # per-core node permutation: greedy LPT (sorted by degree desc, assign to
    # lightest non-full bin) so every 128-node tile has near-equal edge count
    import heapq
    gid = np.empty(n_nodes, np.int64)  # global padded id under new order
    perm_per_core = []
    for p in range(ncores):
        nodes = np.arange(n0[p], n1[p])
        order = nodes[np.argsort(-deg[nodes], kind="stable")]
        slots = np.full(nshard, -1, np.int64)  # slot -> global node (or -1 pad)
        sums = np.zeros(nt, np.int64)
        fill = np.zeros(nt, np.int64)
        heap = [(0, 0, t) for t in range(nt)]
        heapq.heapify(heap)
        for v in order:
            while True:
                _, _, b = heapq.heappop(heap)
                if fill[b] < P:
                    break
            slots[b * P + fill[b]] = v
            sums[b] += deg[v]
            fill[b] += 1
            heapq.heappush(heap, (int(sums[b]), int(fill[b]), b))
        # remaining pad slots stay -1
        real = slots >= 0
        gid[slots[real]] = p * nshard + np.nonzero(real)[0]
        perm_per_core.append(slots)

    # per-core edge lists grouped by dst tile
    dst_core = core_of_node[dst_g]
    dst_lid = gid[dst_g] % nshard          # local new id of dst
    src_gid = gid[src_g]                    # global padded id of src

    # chunks per tile (uniform across cores & tiles)
    cpt = 1
    edges_by = []
    for p in range(ncores):
        m = dst_core == p
        dl, sg = dst_lid[m], src_gid[m]
        t = dl // P
        # sort each tile's edges by source address: gather instructions then
        # read ascending, clustered HBM addresses (order is free - S absorbs it)
        per_tile = []
        for i in range(nt):
            dli, sgi = dl[t == i], sg[t == i]
            o = np.argsort(sgi, kind="stable")
            per_tile.append((dli[o], sgi[o]))
        edges_by.append(per_tile)
        for dli, _ in per_tile:
            cpt = max(cpt, (len(dli) + P - 1) // P)

    nchunk = nt * cpt
    offs_all, s_all, pool_all, inv_all, x_all, deg_all = [], [], [], [], [], []
    for p in range(ncores):
        offs = np.zeros((P, nchunk), np.int32)
        dsts = np.full((P, nchunk), -1.0, np.float32)  # dst slot per edge, -1 pad
        for t in range(nt):
            dl, sg = edges_by[p][t]
            ne = len(dl)
            for c in range(cpt):
                e0, e1 = c * P, min((c + 1) * P, ne)
                k = t * cpt + c
                if e1 > e0:
                    sl = np.arange(e1 - e0)
                    offs[sl, k] = sg[e0:e1]
                    dsts[sl, k] = dl[e0:e1] % P
        offs_all.append(offs)
        s_all.append(dsts)  # fp32: tensor_scalar is_equal needs f32 scalar

        slots = perm_per_core[p]
        gids = np.full((P, nt), -1.0, np.float32)  # local graph id per slot, -1 pad
        counts = np.zeros(gp, np.int64)
        for t in range(nt):
            sl = slots[t * P:(t + 1) * P]
            real = sl >= 0
            g = np.where(real, batch[np.where(real, sl, 0)] - p * gp, -1)
            gids[:, t] = g
        np.add.at(counts, batch[slots[slots >= 0]] - p * gp, 1)
        pool_all.append(gids)
        inv_all.append((1.0 / np.maximum(counts, 1)).astype(np.float32)[:, None])

        xs = np.zeros((nshard, D), np.float32)
        real = slots >= 0
        xs[np.nonzero(real)[0]] = np.asarray(x, np.float32)[slots[real]]
        x_all.append(xs.astype(BF))
        dv = np.zeros(nshard, np.float32)
        dv[np.nonzero(real)[0]] = deg[slots[real]]
        deg_all.append(dv[None, :].astype(BF))

    return dict(nshard=nshard, nt=nt, cpt=cpt, gp=gp, offs=offs_all, S=s_all,
                pool=pool_all, inv=inv_all, x=x_all, deg=deg_all)


def _wpanels(W, b):
    """Split [K, N] weight into two K-halves + bias row, bf16."""
    K = W.shape[0]
    h = K // 2
    return (np.ascontiguousarray(W[:h]).astype(BF),
            np.ascontiguousarray(W[h:]).astype(BF),
            np.asarray(b, np.float32)[None, :].astype(BF))


# ------------------------------------------------------------ device program
def _build(ncores, nshard, nt, cpt, gp, gather_group=5):
    nchunk = nt * cpt
    nfull = ncores * nshard
    nc = bacc.Bacc("TRN2", target_bir_lowering=False, debug=False)

    x_d = nc.dram_tensor("x", [nshard, D], BF16, kind="ExternalInput")
    offs_d = nc.dram_tensor("offs", [P, nchunk], I32, kind="ExternalInput")
    dsts_d = nc.dram_tensor("dsts", [P, nchunk], F32, kind="ExternalInput")
    iota_d = nc.dram_tensor("iota", [P, P], BF16, kind="ExternalInput")
    gids_d = nc.dram_tensor("gids", [P, nt], F32, kind="ExternalInput")
    inv_d = nc.dram_tensor("inv", [gp, 1], F32, kind="ExternalInput")
    wa_d = nc.dram_tensor("Wa", [DH, 4 * D], BF16, kind="ExternalInput")   # fused l1 + gcn2..4, top half
    wb_d = nc.dram_tensor("Wb", [DH, 4 * D], BF16, kind="ExternalInput")   # bottom half
    bias_d = nc.dram_tensor("bias", [1, 4 * D], BF16, kind="ExternalInput")
    c1_d = nc.dram_tensor("c1", [1, D], BF16, kind="ExternalInput")        # emb_b @ gcn_W[0]
    deg_d = nc.dram_tensor("deg", [1, nshard], BF16, kind="ExternalInput")
    w1a_d = nc.dram_tensor("W1a", [DH, DH], BF16, kind="ExternalInput")
    w1b_d = nc.dram_tensor("W1b", [DH, DH], BF16, kind="ExternalInput")
    b1_d = nc.dram_tensor("b1", [1, DH], BF16, kind="ExternalInput")
    w2_d = nc.dram_tensor("W2", [DH, 36], BF16, kind="ExternalInput")
    b2_d = nc.dram_tensor("b2", [1, 36], BF16, kind="ExternalInput")
    w3_d = nc.dram_tensor("W3", [36, 10], BF16, kind="ExternalInput")
    b3_d = nc.dram_tensor("b3", [1, 10], BF16, kind="ExternalInput")
    out_d = nc.dram_tensor("out", [gp, 10], F32, kind="ExternalOutput")

    G = gather_group
    ngroups = (nt + G - 1) // G

    from contextlib import ExitStack
    with tile.TileContext(nc) as tc, ExitStack() as ctx:
        cp = ctx.enter_context(tc.tile_pool(name="const", bufs=1))
        dp = ctx.enter_context(tc.tile_pool(name="dram", bufs=1, space="DRAM"))
        xp = ctx.enter_context(tc.tile_pool(name="xin", bufs=3))
        gbp = ctx.enter_context(tc.tile_pool(name="gbuf", bufs=4))
        asp = ctx.enter_context(tc.tile_pool(name="aggsb", bufs=3))
        atp = ctx.enter_context(tc.tile_pool(name="aggT", bufs=3))
        smp = ctx.enter_context(tc.tile_pool(name="small", bufs=1))
        ptp = ctx.enter_context(tc.tile_pool(name="ptr", bufs=1, space="PSUM"))
        pgp = ctx.enter_context(tc.tile_pool(name="pagg", bufs=2, space="PSUM"))
        php = ctx.enter_context(tc.tile_pool(name="phw", bufs=2, space="PSUM"))
        ppp = ctx.enter_context(tc.tile_pool(name="ppool", bufs=1, space="PSUM"))

        # ---- constants
        dsts_sb = cp.tile([P, nchunk], F32)
        nc.sync.dma_start(dsts_sb[:], dsts_d[:])
        iota_sb = cp.tile([P, P], BF16)
        nc.sync.dma_start(iota_sb[:], iota_d[:])
        s_sb = cp.tile([P, nchunk * P], BF16)
        for k in range(nchunk):
            nc.vector.tensor_scalar(
                out=s_sb[:, k * P:(k + 1) * P], in0=iota_sb[:],
                scalar1=dsts_sb[:, k:k + 1], scalar2=None,
                op0=mybir.AluOpType.is_equal)
        offs_sb = cp.tile([P, nchunk], I32)
        nc.sync.dma_start(offs_sb[:], offs_d[:])
        gids_sb = cp.tile([P, nt], F32)
        nc.sync.dma_start(gids_sb[:], gids_d[:])
        pool_sb = cp.tile([P, nt * 32], BF16)
        for t in range(nt):
            nc.vector.tensor_scalar(
                out=pool_sb[:, t * 32:(t + 1) * 32], in0=iota_sb[:, :32],
                scalar1=gids_sb[:, t:t + 1], scalar2=None,
                op0=mybir.AluOpType.is_equal)
        inv_sb = cp.tile([gp, 1], F32)
        nc.sync.dma_start(inv_sb[:], inv_d[:])
        wa_sb = cp.tile([DH, 4 * D], BF16)
        nc.sync.dma_start(wa_sb[:], wa_d[:])
        wb_sb = cp.tile([DH, 4 * D], BF16)
        nc.sync.dma_start(wb_sb[:], wb_d[:])
        bias_sb = cp.tile([1, 4 * D], BF16)
        nc.sync.dma_start(bias_sb[:], bias_d[:])
        c1_sb = cp.tile([1, D], BF16)
        nc.sync.dma_start(c1_sb[:], c1_d[:])
        deg_sb = cp.tile([1, nshard], BF16)
        nc.sync.dma_start(deg_sb[:], deg_d[:])
        w1a_sb = cp.tile([DH, DH], BF16); nc.sync.dma_start(w1a_sb[:], w1a_d[:])
        w1b_sb = cp.tile([DH, DH], BF16); nc.sync.dma_start(w1b_sb[:], w1b_d[:])
        b1_sb = cp.tile([1, DH], BF16); nc.sync.dma_start(b1_sb[:], b1_d[:])
        w2_sb = cp.tile([DH, 36], BF16); nc.sync.dma_start(w2_sb[:], w2_d[:])
        b2_sb = cp.tile([1, 36], BF16); nc.sync.dma_start(b2_sb[:], b2_d[:])
        w3_sb = cp.tile([36, 10], BF16); nc.sync.dma_start(w3_sb[:], w3_d[:])
        b3_sb = cp.tile([1, 10], BF16); nc.sync.dma_start(b3_sb[:], b3_d[:])

        ident = cp.tile([P, P], BF16)
        make_identity(nc, ident[:])
        ones = cp.tile([1, P], BF16)
        nc.vector.memset(ones[:], 1.0)

        h_sb = cp.tile([P, nt * D], BF16)  # node (t*P+p) at [p, t*D : t*D+D]

        h_bounce = dp.tile([nshard, D], BF16)
        h_full = dp.tile([nfull, D], BF16)
        h_bounce_pv = h_bounce[:].rearrange("(t p) d -> p t d", p=P)
        h_sb_3d = h_sb[:].rearrange("p (t d) -> p t d", d=D)

        def tile_tail(t, psum_hw, layer):
            """relu psum -> h_sb, plus pooling on the last layer"""
            hslice = h_sb[:, t * D:(t + 1) * D]
            nc.scalar.activation(hslice, psum_hw[:],
                                 mybir.ActivationFunctionType.Relu)
            if layer == N_LAYERS:
                nc.tensor.matmul(ppool_t[:], lhsT=pool_sb[:, t * 32:(t + 1) * 32],
                                 rhs=hslice, start=(t == 0), stop=(t == nt - 1))

        def transform(t, aT_a, aT_b, layer):
            """psum_hw = aT.T @ W_layer + b_layer (+ deg*c1 on fused layer 1)"""
            li = layer - 1
            ph = php.tile([P, D], F32, tag="phw")
            nc.tensor.matmul(ph[:], lhsT=aT_a[:], rhs=wa_sb[:, li * D:(li + 1) * D],
                             start=True, stop=False)
            nc.tensor.matmul(ph[:], lhsT=aT_b[:], rhs=wb_sb[:, li * D:(li + 1) * D],
                             start=False, stop=False)
            if layer == 1:
                nc.tensor.matmul(ph[:], lhsT=deg_sb[:1, t * P:(t + 1) * P], rhs=c1_sb[:],
                                 start=False, stop=False)
            nc.tensor.matmul(ph[:], lhsT=ones[:1, :P], rhs=bias_sb[:, li * D:(li + 1) * D],
                             start=False, stop=True)
            return ph

        def transpose_pair(src_sb, m):
            """[m, 146] bf16 -> two [73, m] bf16 panels"""
            outs = []
            for half in range(2):
                pt = ptp.tile([DH, P], BF16, tag="ptr")
                nc.tensor.transpose(pt[:, :m], src_sb[:m, half * DH:(half + 1) * DH],
                                    ident[:m, :m])
                at = atp.tile([DH, P], BF16, tag="aggT")
                nc.vector.tensor_copy(at[:, :m], pt[:, :m])
                outs.append(at)
            return outs

        # ---- embedding folded into layer 1: h_bounce = bf16(x)
        # (cast DRAM->SBUF via SWDGE, then plain SBUF->DRAM; DRAM->DRAM cast faults HW)
        x_pv = x_d[:].rearrange("(t p) d -> p t d", p=P)
        nc.sync.dma_start(h_sb_3d, x_pv)            # 3.7MB load (bf16, host pre-cast)
        nc.sync.dma_start(h_bounce_pv, h_sb_3d)     # 3.7MB store

        # ---- GCN layers
        for layer in range(1, N_LAYERS + 1):
            nc.gpsimd.collective_compute(
                "AllGather", mybir.AluOpType.bypass,
                replica_groups=[list(range(ncores))],
                ins=[h_bounce.opt()], outs=[h_full.opt()],
            )
            if layer == N_LAYERS:
                ppool_t = ppp.tile([32, D], F32)
            for t in range(nt):
                gb = gbp.tile([P, cpt * D], BF16, tag="gbuf")
                for c in range(cpt):
                    k = t * cpt + c
                    nc.gpsimd.indirect_dma_start(
                        out=gb[:, c * D:(c + 1) * D], out_offset=None,
                        in_=h_full[:],
                        in_offset=IndirectOffsetOnAxis(
                            ap=offs_sb[:, k:k + 1], axis=0),
                    )
                # aggT computed directly: aggT_half = sum_c G_c[:, half].T @ S_c
                pta = pgp.tile([DH, P], F32, tag="pagga")
                ptb = pgp.tile([DH, P], F32, tag="paggb")
                for c in range(cpt):
                    k = t * cpt + c
                    nc.tensor.matmul(pta[:], lhsT=gb[:, c * D:c * D + DH],
                                     rhs=s_sb[:, k * P:(k + 1) * P],
                                     start=(c == 0), stop=(c == cpt - 1))
                    nc.tensor.matmul(ptb[:], lhsT=gb[:, c * D + DH:(c + 1) * D],
                                     rhs=s_sb[:, k * P:(k + 1) * P],
                                     start=(c == 0), stop=(c == cpt - 1))
                aa = atp.tile([DH, P], BF16, tag="aggT")
                nc.vector.tensor_copy(aa[:], pta[:])
                ab = atp.tile([DH, P], BF16, tag="aggT")
                nc.vector.tensor_copy(ab[:], ptb[:])
                ph = transform(t, aa, ab, layer)
                tile_tail(t, ph, layer)
            if layer < N_LAYERS:
                nc.sync.dma_start(h_bounce_pv, h_sb_3d)

        # ---- mean pool + MLP
        hg = smp.tile([gp, D], F32, tag="hg")
        nc.vector.tensor_scalar_mul(hg[:], ppool_t[:gp, :], inv_sb[:, :1])
        hgb = smp.tile([gp, D], BF16, tag="hgb")
        nc.vector.tensor_copy(hgb[:], hg[:])

        ga, gbn = transpose_pair(hgb, gp)
        p1 = php.tile([gp, DH], F32, tag="phw")
        nc.tensor.matmul(p1[:], lhsT=ga[:, :gp], rhs=w1a_sb[:], start=True, stop=False)
        nc.tensor.matmul(p1[:], lhsT=gbn[:, :gp], rhs=w1b_sb[:], start=False, stop=False)
        nc.tensor.matmul(p1[:], lhsT=ones[:1, :gp], rhs=b1_sb[:], start=False, stop=True)
        z1 = smp.tile([gp, DH], BF16, tag="z1")
        nc.scalar.activation(z1[:], p1[:], mybir.ActivationFunctionType.Relu)

        ptz = ptp.tile([DH, P], BF16, tag="ptr")
        nc.tensor.transpose(ptz[:, :gp], z1[:, :], ident[:gp, :gp])
        z1t = atp.tile([DH, P], BF16, tag="aggT")
        nc.vector.tensor_copy(z1t[:, :gp], ptz[:, :gp])

        p2 = php.tile([gp, 36], F32, tag="phw")
        nc.tensor.matmul(p2[:], lhsT=z1t[:, :gp], rhs=w2_sb[:], start=True, stop=False)
        nc.tensor.matmul(p2[:], lhsT=ones[:1, :gp], rhs=b2_sb[:], start=False, stop=True)
        z2 = smp.tile([gp, 36], BF16, tag="z2")
        nc.scalar.activation(z2[:], p2[:], mybir.ActivationFunctionType.Relu)

        ptz2 = ptp.tile([36, P], BF16, tag="ptr")
        nc.tensor.transpose(ptz2[:, :gp], z2[:, :], ident[:gp, :gp])
        z2t = atp.tile([36, P], BF16, tag="aggT")
        nc.vector.tensor_copy(z2t[:, :gp], ptz2[:, :gp])

        p3 = php.tile([gp, 10], F32, tag="phw")
        nc.tensor.matmul(p3[:], lhsT=z2t[:36, :gp], rhs=w3_sb[:], start=True, stop=False)
        nc.tensor.matmul(p3[:], lhsT=ones[:1, :gp], rhs=b3_sb[:], start=False, stop=True)
        osb = smp.tile([gp, 10], F32, tag="osb")
        nc.vector.tensor_copy(osb[:], p3[:])
        nc.sync.dma_start(out_d[:], osb[:])

    nc.compile()
    return nc


# ------------------------------------------------------------------- driver
def _in_maps(prep, emb_W, emb_b, gcn_W, gcn_b,
             r_W1, r_b1, r_W2, r_b2, r_W3, r_b3, ncores):
    emb_W = np.asarray(emb_W, np.float32); emb_b = np.asarray(emb_b, np.float32)
    gcn_W = np.asarray(gcn_W, np.float32); gcn_b = np.asarray(gcn_b, np.float32)
    wf1 = emb_W @ gcn_W[0]                       # fused layer-1 weight
    c1 = (emb_b @ gcn_W[0])[None, :].astype(BF)  # deg-scaled bias row
    was, wbs, bs = [], [], []
    for W, b in [(wf1, gcn_b[0])] + [(gcn_W[i], gcn_b[i]) for i in range(1, N_LAYERS)]:
        a, bb, br = _wpanels(W, np.asarray(b, np.float32))
        was.append(a); wbs.append(bb); bs.append(br)
    wa = np.concatenate(was, axis=1)
    wb = np.concatenate(wbs, axis=1)
    bias = np.concatenate(bs, axis=1)
    w1a, w1b, b1 = _wpanels(np.asarray(r_W1, np.float32), np.asarray(r_b1, np.float32))
    common = dict(
        Wa=wa, Wb=wb, bias=bias, c1=c1, W1a=w1a, W1b=w1b, b1=b1,
        W2=np.asarray(r_W2, np.float32).astype(BF), b2=np.asarray(r_b2, np.float32)[None].astype(BF),
        W3=np.asarray(r_W3, np.float32).astype(BF), b3=np.asarray(r_b3, np.float32)[None].astype(BF),
    )
    iota = np.broadcast_to(np.arange(P, dtype=np.float32), (P, P)).astype(BF).copy()
    return [dict(x=prep["x"][p], offs=prep["offs"][p], dsts=prep["S"][p], iota=iota,
                 gids=prep["pool"][p], inv=prep["inv"][p], deg=prep["deg"][p], **common)
            for p in range(ncores)]


_CACHE = {}


def kernel(x, edge_index, batch, emb_W, emb_b, gcn_W, gcn_b,
           r_W1, r_b1, r_W2, r_b2, r_W3, r_b3):
    ncores = 8
    prep = _prep(np.asarray(x), np.asarray(edge_index), np.asarray(batch), ncores)
    key = (ncores, prep["nshard"], prep["cpt"])
    if key not in _CACHE:
        _CACHE[key] = _build(ncores, prep["nshard"], prep["nt"], prep["cpt"], prep["gp"])
    nc = _CACHE[key]
    maps = _in_maps(prep, emb_W, emb_b, gcn_W, gcn_b,
                    r_W1, r_b1, r_W2, r_b2, r_W3, r_b3, ncores)
    res = run_bass_kernel_spmd(nc, maps, core_ids=list(range(ncores)))
    return np.concatenate([res.results[p]["out"] for p in range(ncores)], axis=0)

